# revision 26
# baseline (speedup 1.0000x reference)
"""GatedDeltaNet Trainium2 kernel (8 NeuronCores, SPMD).

Shapes: B=2, S=4096, D=2048, HK=HV=16, DK=DV=128, KCONV=4.
Sharding: core c -> batch b=c//4, heads h0=4*(c%4) .. h0+4 (batch x head
parallel).  Each core computes its 4 heads' full pipeline and a partial
out-projection [D, S]; the host sums 4 partials per batch and transposes.

Device algorithm (per core), all activations channel-major [chan, time]:
  P1  fused projection: mixed/z/beta/alpha = W_all^T @ hs^T  (bf16 matmuls)
  P2  depthwise causal conv (4 taps) + SiLU; l2-norm of q,k (partition-dim
      sums via ones-matmul); per-head decay stats (cumsum via DVE scan)
  P3  chunked gated delta rule, chunk C=128:
        M[t,s] = beta_t (k_t.k_s) exp(gc_t-gc_s) (s<t);  B = -M
        [W|U] = (I+M)^{-1} [beta*Gam*K | beta*V] via product
                 prod_j (I + B^{2^j}) (B nilpotent, 7 levels, all matmuls)
        O = P@U + (Gam*Q - P@W) @ S0,  P[t,s] = (q_t.k_s) exp(gc_t-gc_s) (s<=t)
        S' = (gtot*I - K'^T W) @ S0 + K'^T U,  K'_t = exp(gc_C-gc_t) k_t
      then gated RMSNorm and gate with silu(z)
  P4  row-parallel out-projection partial: out^T += W_out_slice^T @ core^T
"""

import numpy as np
import ml_dtypes

B, S, D = 2, 4096, 2048
HK, HV, DK, DV, KCONV = 16, 16, 128, 128, 4
KEY_DIM, VALUE_DIM = HK * DK, HV * DV
CONV_DIM = 2 * KEY_DIM + VALUE_DIM
EPS = 1e-6
HPC = 4            # heads per core
C = 128            # chunk length
NCHUNK = S // C    # 32
NT = S // 512      # 8 token blocks
KT = D // 128      # 16 k-tiles
CT_Q, CT_K, CT_V, CT_Z = 0, 4, 8, 12   # ctile index bases
NCT = 16           # 16 x 128 cols (q|k|v|z)
COLS = 2048

BF16 = ml_dtypes.bfloat16
NEG = -1e30

_CACHE = {}


def build_nc(debug=False):
    import concourse.bass as bass
    import concourse.mybir as mybir
    import concourse.tile as tile
    from concourse import bacc

    fp32 = mybir.dt.float32
    bf16 = mybir.dt.bfloat16
    AF = mybir.ActivationFunctionType
    OP = mybir.AluOpType
    AX = mybir.AxisListType

    nc = bacc.Bacc("TRN2", target_bir_lowering=False, debug=False)

    hsT = nc.dram_tensor("hsT", [D, S], bf16, kind="ExternalInput")
    W_all = nc.dram_tensor("W_all", [D, COLS], bf16, kind="ExternalInput")
    convw = nc.dram_tensor("convw", [12 * 128, KCONV], fp32, kind="ExternalInput")
    growm_d = nc.dram_tensor("growm", [2 * HPC, S], fp32, kind="ExternalInput")
    colst_d = nc.dram_tensor("colstats", [128, HPC * 192], fp32, kind="ExternalInput")
    normw_d = nc.dram_tensor("normw", [128, 1], fp32, kind="ExternalInput")
    W_out_d = nc.dram_tensor("W_out", [HPC * DV, D], bf16, kind="ExternalInput")
    masks_d = nc.dram_tensor("masks", [128, 384], fp32, kind="ExternalInput")
    identb_d = nc.dram_tensor("identb", [128, 128], bf16, kind="ExternalInput")
    identf_d = nc.dram_tensor("identf", [128, 128], fp32, kind="ExternalInput")
    outT = nc.dram_tensor("outT", [D, S], bf16, kind="ExternalOutput")

    dbg = {}
    if debug:
        for nm, shp in [("d_mixed", [1536, S]), ("d_qg", [512, S]),
                        ("d_kh", [512, S]), ("d_vc", [512, S]),
                        ("d_z", [512, S]), ("d_core", [512, S])]:
            dbg[nm] = nc.dram_tensor(nm, shp, bf16, kind="ExternalOutput")

    from contextlib import ExitStack
    with tile.TileContext(nc) as tc, ExitStack() as ctx:
        cst = ctx.enter_context(tc.tile_pool(name="cst", bufs=1))
        dp = ctx.enter_context(tc.tile_pool(name="dram", bufs=1, space="DRAM"))

        mixed_d = dp.tile([1536, S], bf16, tag="mixed_d")
        z_d = dp.tile([512, S], bf16, tag="z_d")
        qg_d = dp.tile([512, S], bf16, tag="qg_d")
        kh_d = dp.tile([512, S], bf16, tag="kh_d")
        vc_d = dp.tile([512, S], bf16, tag="vc_d")
        core_d = dp.tile([512, S], bf16, tag="core_d")

        # constants
        masks = cst.tile([128, 384], fp32, tag="masks")
        nc.sync.dma_start(out=masks, in_=masks_d.ap())
        maskLS, maskUS, maskUI = masks[:, 0:128], masks[:, 128:256], masks[:, 256:384]
        identb = cst.tile([128, 128], bf16, tag="identb")
        nc.sync.dma_start(out=identb, in_=identb_d.ap())
        identf = cst.tile([128, 128], fp32, tag="identf")
        nc.sync.dma_start(out=identf, in_=identf_d.ap())
        normw = cst.tile([128, 1], fp32, tag="normw")
        nc.sync.dma_start(out=normw, in_=normw_d.ap())
        onesb = cst.tile([128, 1], bf16, tag="onesb")
        nc.vector.memset(onesb, 1.0)
        colst = cst.tile([128, HPC * 192], fp32, tag="colst")
        nc.sync.dma_start(out=colst, in_=colst_d.ap())
        hstat = {}
        for h in range(HPC):
            o = h * 192
            hstat[h] = dict(
                gcT=colst[:, o:o + 32], bT=colst[:, o + 32:o + 64],
                nbT=colst[:, o + 64:o + 96], grevT=colst[:, o + 96:o + 128],
                bgT=colst[:, o + 128:o + 160], gtotT=colst[:, o + 160:o + 192])

        # ---------------- Phase 1: projections ----------------
        with tc.tile_pool(name="p1w", bufs=1) as wp, \
             tc.tile_pool(name="p1", bufs=3) as p1, \
             tc.tile_pool(name="p1h", bufs=2) as p1h, \
             tc.tile_pool(name="pp1", bufs=4, space="PSUM") as pp1:
            walls = wp.tile([128, KT, COLS], bf16, tag="walls")
            nc.sync.dma_start(
                out=walls, in_=W_all.ap().rearrange("(kt p) c -> p kt c", p=128))
            for nt in range(NT):
                ts = slice(nt * 512, (nt + 1) * 512)
                hsblk = p1h.tile([128, KT, 512], bf16, tag="hsblk")
                nc.sync.dma_start(
                    out=hsblk,
                    in_=hsT.ap().rearrange("(kt p) t -> p kt t", p=128)[:, :, ts])
                for ct in range(NCT):
                    c0 = ct * 128
                    ps = pp1.tile([128, 512], fp32, tag="ps")
                    for k in range(KT):
                        nc.tensor.matmul(
                            ps, walls[:, k, c0:c0 + 128], hsblk[:, k, :],
                            start=(k == 0), stop=(k == KT - 1))
                    t = p1.tile([128, 512], bf16, tag="t")
                    if ct < 12:
                        nc.vector.tensor_copy(out=t, in_=ps)
                        nc.sync.dma_start(out=mixed_d[c0:c0 + 128, ts], in_=t)
                    else:
                        nc.scalar.activation(out=t, in_=ps, func=AF.Silu)
                        nc.sync.dma_start(
                            out=z_d[(ct - CT_Z) * 128:(ct - CT_Z) * 128 + 128, ts],
                            in_=t)

        # ---------------- Phase 2b: conv + silu + l2norm ----------------
        epsq = cst.tile([1, 1], fp32, tag="epsq")
        nc.vector.memset(epsq, EPS * 128.0)
        epsk = cst.tile([1, 1], fp32, tag="epsk")
        nc.vector.memset(epsk, EPS)
        epsO = cst.tile([128, 1], fp32, tag="epsO")
        nc.vector.memset(epsO, EPS)

        with tc.tile_pool(name="p2b", bufs=2) as p2, \
             tc.tile_pool(name="p2r", bufs=2) as p2r, \
             tc.tile_pool(name="pp2", bufs=8, space="PSUM") as pp2:
            for ct in range(12):
                c0 = ct * 128
                pl = p2.tile([128, S], bf16, tag="pl")
                nc.sync.dma_start(out=pl, in_=mixed_d[c0:c0 + 128, :])
                cwt = p2r.tile([128, KCONV], fp32, tag="cwt")
                nc.sync.dma_start(out=cwt, in_=convw.ap()[c0:c0 + 128, :])
                acc = p2.tile([128, S], bf16, tag="acc")
                nc.vector.tensor_scalar_mul(out=acc, in0=pl, scalar1=cwt[:, 3:4])
                for j in range(3):
                    sh = 3 - j
                    nc.vector.scalar_tensor_tensor(
                        out=acc[:, sh:], in0=pl[:, :S - sh], scalar=cwt[:, j:j + 1],
                        in1=acc[:, sh:], op0=mybir.AluOpType.mult,
                        op1=mybir.AluOpType.add)
                sil = p2.tile([128, S], bf16, tag="sil")
                nc.scalar.activation(out=sil, in_=acc, func=AF.Silu)
                if ct < 8:  # q or k: l2 normalize over dk (partition dim)
                    isq = ct < 4
                    h = ct if isq else ct - 4
                    sq = p2.tile([128, S], bf16, tag="sq")
                    nc.scalar.activation(out=sq, in_=sil, func=AF.Square)
                    srow = p2r.tile([1, S], fp32, tag="srow")
                    for nt in range(NT):
                        ts = slice(nt * 512, (nt + 1) * 512)
                        pss = pp2.tile([1, 512], fp32, tag="pss")
                        nc.tensor.matmul(pss, onesb, sq[:, ts], start=True, stop=True)
                        nc.scalar.activation(
                            out=srow[:, ts], in_=pss, func=AF.Sqrt,
                            bias=epsq if isq else epsk,
                            scale=128.0 if isq else 1.0)
                    nc.vector.reciprocal(out=srow, in_=srow)
                    rrow2 = p2r.tile([1, S], bf16, tag="rrow2")
                    nc.vector.tensor_copy(out=rrow2, in_=srow)
                    brd = p2.tile([128, S], bf16, tag="brd")
                    nc.gpsimd.partition_broadcast(brd, rrow2)
                    opl = p2.tile([128, S], bf16, tag="opl")
                    nc.vector.tensor_mul(out=opl, in0=sil, in1=brd)
                    dst = qg_d if isq else kh_d
                    nc.sync.dma_start(out=dst[h * 128:h * 128 + 128, :], in_=opl)
                    if debug:
                        dd = dbg["d_qg"] if isq else dbg["d_kh"]
                        nc.sync.dma_start(out=dd.ap()[h * 128:h * 128 + 128, :],
                                          in_=opl)
                else:
                    nc.sync.dma_start(
                        out=vc_d[(ct - 8) * 128:(ct - 8) * 128 + 128, :], in_=sil)
                    if debug:
                        nc.sync.dma_start(
                            out=dbg["d_vc"].ap()[(ct - 8) * 128:(ct - 8) * 128 + 128, :],
                            in_=sil)
        if debug:
            with tc.tile_pool(name="dbgp", bufs=2) as dbp:
                for ct in range(12):
                    t = dbp.tile([128, S], bf16, tag="t")
                    nc.sync.dma_start(out=t, in_=mixed_d[ct * 128:(ct + 1) * 128, :])
                    nc.sync.dma_start(out=dbg["d_mixed"].ap()[ct * 128:(ct + 1) * 128, :], in_=t)
                for ct in range(4):
                    t = dbp.tile([128, S], bf16, tag="t")
                    nc.sync.dma_start(out=t, in_=z_d[ct * 128:(ct + 1) * 128, :])
                    nc.sync.dma_start(out=dbg["d_z"].ap()[ct * 128:(ct + 1) * 128, :], in_=t)

        # ---------------- Phase 3: chunked gated delta rule ----------------
        with tc.tile_pool(name="p3pl", bufs=2) as p3p, \
             tc.tile_pool(name="p3row", bufs=1) as p3r, \
             tc.tile_pool(name="p3brd", bufs=1) as p3b, \
             tc.tile_pool(name="p3w", bufs=3) as p3, \
             tc.tile_pool(name="p3sq", bufs=4) as p3s, \
             tc.tile_pool(name="p3st", bufs=3) as p3st, \
             tc.tile_pool(name="ppA", bufs=1, space="PSUM") as ppA, \
             tc.tile_pool(name="ppB", bufs=1, space="PSUM") as ppB, \
             tc.tile_pool(name="ppT", bufs=1, space="PSUM") as ppT, \
             tc.tile_pool(name="ppX", bufs=1, space="PSUM") as ppX, \
             tc.tile_pool(name="ppS", bufs=1, space="PSUM") as ppS:
            for h in range(HPC):
                st = hstat[h]
                qp = p3p.tile([128, S], bf16, tag="qp")
                nc.sync.dma_start(out=qp, in_=qg_d[h * 128:h * 128 + 128, :])
                kp = p3p.tile([128, S], bf16, tag="kp")
                nc.sync.dma_start(out=kp, in_=kh_d[h * 128:h * 128 + 128, :])
                vp = p3p.tile([128, S], bf16, tag="vp")
                nc.sync.dma_start(out=vp, in_=vc_d[h * 128:h * 128 + 128, :])
                szp = p3p.tile([128, S], bf16, tag="szp")
                nc.sync.dma_start(out=szp, in_=z_d[h * 128:h * 128 + 128, :])
                cp = p3p.tile([128, S], bf16, tag="cp")
                gcrow = p3r.tile([1, S], fp32, tag="gcrow")
                nc.sync.dma_start(out=gcrow, in_=growm_d.ap()[h:h + 1, :])
                ngcrow = p3r.tile([1, S], fp32, tag="ngcrow")
                nc.sync.dma_start(out=ngcrow, in_=growm_d.ap()[HPC + h:HPC + h + 1, :])
                gbrdF = p3b.tile([128, S], fp32, tag="gbrdF")
                nc.gpsimd.partition_broadcast(gbrdF, gcrow)
                ngbrdF = p3b.tile([128, S], fp32, tag="ngbrdF")
                nc.gpsimd.partition_broadcast(ngbrdF, ngcrow)
                Scur = p3st.tile([128, 128], bf16, tag="S")
                nc.vector.memset(Scur, 0.0)
                for c in range(NCHUNK):
                    cs = slice(c * C, (c + 1) * C)
                    col = slice(c, c + 1)
                    gbrd = gbrdF[:, cs]
                    ngbrd = ngbrdF[:, cs]
                    # decay matrices
                    dS = p3.tile([128, 128], fp32, tag="dS")
                    nc.vector.scalar_tensor_tensor(
                        out=dS, in0=ngbrd, scalar=st["gcT"][:, col], in1=maskLS,
                        op0=OP.add, op1=OP.add)
                    nc.scalar.activation(out=dS, in_=dS, func=AF.Exp)
                    dST = p3.tile([128, 128], fp32, tag="dST")
                    nc.vector.scalar_tensor_tensor(
                        out=dST, in0=gbrd, scalar=st["gcT"][:, col], in1=maskUS,
                        op0=OP.subtract, op1=OP.add)
                    nc.scalar.activation(out=dST, in_=dST, func=AF.Exp)
                    dIT = p3.tile([128, 128], fp32, tag="dIT")
                    nc.vector.scalar_tensor_tensor(
                        out=dIT, in0=gbrd, scalar=st["gcT"][:, col], in1=maskUI,
                        op0=OP.subtract, op1=OP.add)
                    nc.scalar.activation(out=dIT, in_=dIT, func=AF.Exp)
                    # KK^T and KQ^T
                    psA = ppA.tile([128, 256], fp32, tag="psA")
                    nc.tensor.matmul(psA[:, 0:128], kp[:, cs], kp[:, cs],
                                     start=True, stop=True)
                    nc.tensor.matmul(psA[:, 128:256], kp[:, cs], qp[:, cs],
                                     start=True, stop=True)
                    B0 = p3.tile([128, 128], bf16, tag="B0")
                    nc.vector.scalar_tensor_tensor(
                        out=B0, in0=psA[:, 0:128], scalar=st["nbT"][:, col],
                        in1=dS, op0=OP.mult, op1=OP.mult)
                    # transposes: B^T, K_tm, V_tm
                    psT = ppT.tile([128, 384], bf16, tag="psT")
                    nc.tensor.transpose(psT[:, 0:128], B0, identb)
                    BT0 = p3.tile([128, 128], bf16, tag="BT0")
                    nc.vector.tensor_copy(out=BT0, in_=psT[:, 0:128])
                    nc.tensor.transpose(psT[:, 128:256], kp[:, cs], identb)
                    Ktm = p3.tile([128, 128], bf16, tag="Ktm")
                    nc.vector.tensor_copy(out=Ktm, in_=psT[:, 128:256])
                    nc.tensor.transpose(psT[:, 256:384], vp[:, cs], identb)
                    X = p3s.tile([128, 256], bf16, tag="X")
                    nc.vector.tensor_scalar_mul(
                        out=X[:, 0:128], in0=Ktm, scalar1=st["bgT"][:, col])
                    nc.vector.tensor_scalar_mul(
                        out=X[:, 128:256], in0=psT[:, 256:384],
                        scalar1=st["bT"][:, col])
                    Kpr = p3.tile([128, 128], bf16, tag="Kpr")
                    nc.vector.tensor_scalar_mul(
                        out=Kpr, in0=Ktm, scalar1=st["grevT"][:, col])
                    # Neumann product chain
                    Sq, SqT = B0, BT0
                    for j in range(7):
                        psX = ppX.tile([128, 256], fp32, tag="psX")
                        nc.tensor.matmul(psX, SqT, X, start=True, stop=True)
                        Xn = p3s.tile([128, 256], bf16, tag="X")
                        nc.vector.tensor_add(out=Xn, in0=X, in1=psX)
                        X = Xn
                        if j < 6:
                            psq = ppX.tile([128, 256], fp32, tag="psq")
                            nSqT = p3s.tile([128, 128], bf16, tag="nSqT")
                            nc.tensor.matmul(psq[:, 128:256], Sq, SqT,
                                             start=True, stop=True)
                            nc.vector.tensor_copy(out=nSqT, in_=psq[:, 128:256])
                            if j < 5:
                                nSq = p3s.tile([128, 128], bf16, tag="nSq")
                                nc.tensor.matmul(psq[:, 0:128], SqT, Sq,
                                                 start=True, stop=True)
                                nc.vector.tensor_copy(out=nSq, in_=psq[:, 0:128])
                                Sq = nSq
                            SqT = nSqT
                    # P^T, W^T P^T, H^T
                    PT = p3.tile([128, 128], bf16, tag="PT")
                    nc.vector.tensor_mul(out=PT, in0=psA[:, 128:256], in1=dIT)
                    psB = ppB.tile([128, 256], fp32, tag="psB")
                    nc.tensor.matmul(psB[:, 0:128], X[:, 0:128], PT,
                                     start=True, stop=True)
                    gamB = p3.tile([128, 128], fp32, tag="gamB")
                    nc.scalar.activation(out=gamB, in_=gbrd, func=AF.Exp)
                    QtG = p3.tile([128, 128], bf16, tag="QtG")
                    nc.vector.tensor_mul(out=QtG, in0=qp[:, cs], in1=gamB)
                    QtT = p3.tile([128, 128], bf16, tag="QtT")
                    nc.vector.tensor_sub(out=QtT, in0=QtG, in1=psB[:, 0:128])
                    nc.tensor.matmul(psB[:, 128:256], X[:, 0:128], Kpr,
                                     start=True, stop=True)
                    GT = p3.tile([128, 128], bf16, tag="GT")
                    nc.vector.scalar_tensor_tensor(
                        out=GT, in0=identf, scalar=st["gtotT"][:, col],
                        in1=psB[:, 128:256], op0=OP.mult, op1=OP.subtract)
                    # O and state update
                    psS = ppS.tile([128, 256], fp32, tag="psS")
                    nc.tensor.matmul(psS[:, 0:128], PT, X[:, 128:256],
                                     start=True, stop=False)
                    nc.tensor.matmul(psS[:, 0:128], QtT, Scur,
                                     start=False, stop=True)
                    nc.tensor.matmul(psS[:, 128:256], GT, Scur,
                                     start=True, stop=False)
                    nc.tensor.matmul(psS[:, 128:256], Kpr, X[:, 128:256],
                                     start=False, stop=True)
                    Snew = p3st.tile([128, 128], bf16, tag="S")
                    nc.vector.tensor_copy(out=Snew, in_=psS[:, 128:256])
                    # gated RMS norm of O, transpose back, gate with silu(z)
                    osq = p3.tile([128, 128], bf16, tag="osq")
                    ssum = p3.tile([128, 1], fp32, tag="ssum")
                    nc.scalar.activation(out=osq, in_=psS[:, 0:128], func=AF.Square,
                                         accum_out=ssum)
                    sr = p3.tile([128, 1], fp32, tag="sr")
                    nc.scalar.activation(out=sr, in_=ssum, func=AF.Sqrt,
                                         bias=epsO, scale=1.0 / 128.0)
                    rr = p3.tile([128, 1], fp32, tag="rr")
                    nc.vector.reciprocal(out=rr, in_=sr)
                    otm = p3.tile([128, 128], bf16, tag="otm")
                    nc.vector.tensor_scalar_mul(out=otm, in0=psS[:, 0:128],
                                                scalar1=rr)
                    psO = ppT.tile([128, 128], bf16, tag="psO")
                    nc.tensor.transpose(psO, otm, identb)
                    nc.vector.scalar_tensor_tensor(
                        out=cp[:, cs], in0=psO, scalar=normw, in1=szp[:, cs],
                        op0=OP.mult, op1=OP.mult)
                    Scur = Snew
                nc.sync.dma_start(out=core_d[h * 128:h * 128 + 128, :], in_=cp)
                if debug:
                    nc.sync.dma_start(
                        out=dbg["d_core"].ap()[h * 128:h * 128 + 128, :], in_=cp)

        # ---------------- Phase 4: out projection ----------------
        with tc.tile_pool(name="p4w", bufs=1) as p4w, \
             tc.tile_pool(name="p4", bufs=3) as p4, \
             tc.tile_pool(name="pp4", bufs=4, space="PSUM") as pp4:
            wot = p4w.tile([128, HPC, D], bf16, tag="wot")
            nc.sync.dma_start(
                out=wot, in_=W_out_d.ap().rearrange("(j p) d -> p j d", p=128))
            for nt in range(NT):
                ts = slice(nt * 512, (nt + 1) * 512)
                cblk = p4.tile([128, HPC, 512], bf16, tag="cblk")
                nc.sync.dma_start(
                    out=cblk,
                    in_=core_d.rearrange("(j p) t -> p j t", p=128)[:, :, ts])
                for dt in range(16):
                    ps = pp4.tile([128, 512], fp32, tag="ps")
                    for j in range(HPC):
                        nc.tensor.matmul(ps, wot[:, j, dt * 128:(dt + 1) * 128],
                                         cblk[:, j, :],
                                         start=(j == 0), stop=(j == HPC - 1))
                    ot = p4.tile([128, 512], bf16, tag="ot")
                    nc.vector.tensor_copy(out=ot, in_=ps)
                    nc.sync.dma_start(out=outT.ap()[dt * 128:(dt + 1) * 128, ts],
                                      in_=ot)

    nc.compile()
    return nc


def _host_inputs(hidden_states, W_qkv, W_z, W_b, W_a, conv_w, A_log, dt_bias,
                 norm_w, W_out):
    """Per-core input maps.  beta/g decay stats computed on host (tiny)."""
    masks = np.zeros((128, 384), np.float32)
    r = np.arange(128)
    masks[:, 0:128] = np.where(r[None, :] < r[:, None], 0.0, NEG)     # s<t
    masks[:, 128:256] = np.where(r[None, :] > r[:, None], 0.0, NEG)   # f>p
    masks[:, 256:384] = np.where(r[None, :] >= r[:, None], 0.0, NEG)  # f>=p
    identity = np.eye(128, dtype=np.float32)

    hs2 = hidden_states.reshape(B * S, D).astype(np.float32)
    bet = 1.0 / (1.0 + np.exp(-(hs2 @ W_b)))                    # [B*S, 16]
    g = (-np.exp(A_log.astype(np.float32))[None, :]
         * np.logaddexp(0.0, hs2 @ W_a + dt_bias[None, :]))     # [B*S, 16]
    bet = bet.reshape(B, NCHUNK, C, HK)
    g = g.reshape(B, NCHUNK, C, HK)
    gc_all = np.cumsum(g, axis=2)                               # [B, nc, C, H]

    in_maps = []
    for c in range(8):
        b = c // 4
        h0 = 4 * (c % 4)
        ch = slice(h0 * 128, (h0 + 4) * 128)
        W_all = np.concatenate([
            W_qkv[:, h0 * 128:(h0 + 4) * 128],
            W_qkv[:, KEY_DIM + h0 * 128:KEY_DIM + (h0 + 4) * 128],
            W_qkv[:, 2 * KEY_DIM + h0 * 128:2 * KEY_DIM + (h0 + 4) * 128],
            W_z[:, h0 * 128:(h0 + 4) * 128],
        ], axis=1).astype(BF16)
        cw = np.concatenate([
            conv_w[h0 * 128:(h0 + 4) * 128, 0, :],
            conv_w[KEY_DIM + h0 * 128:KEY_DIM + (h0 + 4) * 128, 0, :],
            conv_w[2 * KEY_DIM + h0 * 128:2 * KEY_DIM + (h0 + 4) * 128, 0, :],
        ], axis=0).astype(np.float32)

        growm = np.zeros((2 * HPC, S), np.float32)
        colst = np.zeros((128, HPC * 192), np.float32)
        for j in range(HPC):
            gc = gc_all[b, :, :, h0 + j]                        # [nc, C]
            be = bet[b, :, :, h0 + j]
            gam = np.exp(gc)
            growm[j] = gc.reshape(S)
            growm[HPC + j] = -gc.reshape(S)
            o = j * 192
            colst[:, o:o + 32] = gc.T                           # gcT
            colst[:, o + 32:o + 64] = be.T                      # bT
            colst[:, o + 64:o + 96] = -be.T                     # nbT
            colst[:, o + 96:o + 128] = np.exp(gc[:, -1][None, :] - gc.T)  # grevT
            colst[:, o + 128:o + 160] = (be * gam).T            # bgT
            colst[:, o + 160:o + 192] = np.exp(gc[:, -1])[None, :] * np.ones((128, 1), np.float32)  # gtotT
        in_maps.append({
            "hsT": np.ascontiguousarray(hidden_states[b].T).astype(BF16),
            "W_all": W_all,
            "convw": np.ascontiguousarray(cw),
            "growm": growm,
            "colstats": colst,
            "normw": norm_w.astype(np.float32).reshape(128, 1),
            "W_out": W_out[ch, :].astype(BF16),
            "masks": masks,
            "identb": identity.astype(BF16),
            "identf": identity,
        })
    return in_maps


def kernel(hidden_states, W_qkv, W_z, W_b, W_a, conv_w, A_log, dt_bias,
           norm_w, W_out):
    from concourse import bass_utils

    if "nc" not in _CACHE:
        _CACHE["nc"] = build_nc(debug=False)
    nc = _CACHE["nc"]
    in_maps = _host_inputs(hidden_states, W_qkv, W_z, W_b, W_a, conv_w,
                           A_log, dt_bias, norm_w, W_out)
    res = bass_utils.run_bass_kernel_spmd(nc, in_maps, core_ids=list(range(8)))
    out = np.zeros((B, S, D), np.float32)
    for c in range(8):
        b = c // 4
        out[b] += res.results[c]["outT"].astype(np.float32).T
    return out


# revision 30
# speedup vs baseline: 20.1578x; 20.1578x over previous
"""GatedDeltaNet Trainium2 kernel (8 NeuronCores, SPMD).

Shapes: B=2, S=4096, D=2048, HK=HV=16, DK=DV=128, KCONV=4.
Sharding: core c -> batch b=c//4, heads h0=4*(c%4) .. h0+4 (batch x head
parallel).  Each core computes its 4 heads' full pipeline and a partial
out-projection [D, S]; the host sums 4 partials per batch and transposes.

Device algorithm (per core), all activations channel-major [chan, time]:
  P1  fused projection: mixed/z/beta/alpha = W_all^T @ hs^T  (bf16 matmuls)
  P2  depthwise causal conv (4 taps) + SiLU; l2-norm of q,k (partition-dim
      sums via ones-matmul); per-head decay stats (cumsum via DVE scan)
  P3  chunked gated delta rule, chunk C=128:
        M[t,s] = beta_t (k_t.k_s) exp(gc_t-gc_s) (s<t);  B = -M
        [W|U] = (I+M)^{-1} [beta*Gam*K | beta*V] via product
                 prod_j (I + B^{2^j}) (B nilpotent, 7 levels, all matmuls)
        O = P@U + (Gam*Q - P@W) @ S0,  P[t,s] = (q_t.k_s) exp(gc_t-gc_s) (s<=t)
        S' = (gtot*I - K'^T W) @ S0 + K'^T U,  K'_t = exp(gc_C-gc_t) k_t
      then gated RMSNorm and gate with silu(z)
  P4  row-parallel out-projection partial: out^T += W_out_slice^T @ core^T
"""

import os
import numpy as np
import ml_dtypes

B, S, D = 2, 4096, 2048
HK, HV, DK, DV, KCONV = 16, 16, 128, 128, 4
KEY_DIM, VALUE_DIM = HK * DK, HV * DV
CONV_DIM = 2 * KEY_DIM + VALUE_DIM
EPS = 1e-6
HPC = 4            # heads per core
C = 128            # chunk length
NCHUNK = S // C    # 32
NT = S // 512      # 8 token blocks
KT = D // 128      # 16 k-tiles
CT_Q, CT_K, CT_V, CT_Z = 0, 4, 8, 12   # ctile index bases
NCT = 16           # 16 x 128 cols (q|k|v|z)
COLS = 2048

BF16 = ml_dtypes.bfloat16
NEG = -1e30

_CACHE = {}


def build_nc(debug=False):
    import concourse.bass as bass
    import concourse.mybir as mybir
    import concourse.tile as tile
    from concourse import bacc

    fp32 = mybir.dt.float32
    bf16 = mybir.dt.bfloat16
    AF = mybir.ActivationFunctionType
    OP = mybir.AluOpType
    AX = mybir.AxisListType

    nc = bacc.Bacc("TRN2", target_bir_lowering=False, debug=False, num_devices=8)

    G4 = [[0, 1, 2, 3], [4, 5, 6, 7]]       # batch groups (token AG, out RS)
    G2 = [[0, 4], [1, 5], [2, 6], [3, 7]]   # head-group pairs (weight AG)

    hsQ = nc.dram_tensor("hsQ", [D, S // 4], bf16, kind="ExternalInput")
    W_half = nc.dram_tensor("W_half", [D // 2, COLS], bf16, kind="ExternalInput")
    Wo_half = nc.dram_tensor("Wo_half", [HPC * DV // 2, D], bf16,
                             kind="ExternalInput")
    hsb = nc.dram_tensor("hsb", [D, S // 4], bf16)
    hsg = nc.dram_tensor("hsg", [4 * D, S // 4], bf16)
    wb = nc.dram_tensor("wb", [D // 2, COLS], bf16)
    wg = nc.dram_tensor("wg", [D, COLS], bf16)
    wob = nc.dram_tensor("wob", [HPC * DV // 2, D], bf16)
    wog = nc.dram_tensor("wog", [HPC * DV, D], bf16)
    rs_in = nc.dram_tensor("rs_in", [D, S], bf16)
    rs_out = nc.dram_tensor("rs_out", [D // 4, S], bf16)
    convw = nc.dram_tensor("convw", [12 * 128, KCONV], fp32, kind="ExternalInput")
    growm_d = nc.dram_tensor("growm", [2 * HPC, S], fp32, kind="ExternalInput")
    colst_d = nc.dram_tensor("colstats", [128, HPC * 192], fp32, kind="ExternalInput")
    normw_d = nc.dram_tensor("normw", [128, 1], fp32, kind="ExternalInput")
    masks_d = nc.dram_tensor("masks", [128, 384], fp32, kind="ExternalInput")
    identb_d = nc.dram_tensor("identb", [128, 128], bf16, kind="ExternalInput")
    identf_d = nc.dram_tensor("identf", [128, 128], fp32, kind="ExternalInput")
    outR = nc.dram_tensor("outR", [D // 4, S], bf16, kind="ExternalOutput")

    dbg = {}
    if debug:
        for nm, shp in [("d_mixed", [1536, S]), ("d_qg", [512, S]),
                        ("d_kh", [512, S]), ("d_vc", [512, S]),
                        ("d_z", [512, S]), ("d_core", [512, S])]:
            dbg[nm] = nc.dram_tensor(nm, shp, bf16, kind="ExternalOutput")

    from contextlib import ExitStack
    with tile.TileContext(nc) as tc, ExitStack() as ctx:
        cst = ctx.enter_context(tc.tile_pool(name="cst", bufs=1))
        dp = ctx.enter_context(tc.tile_pool(name="dram", bufs=1, space="DRAM"))

        mixed_d = dp.tile([1536, S], bf16, tag="mixed_d")
        z_d = dp.tile([512, S], bf16, tag="z_d")
        qg_d = dp.tile([512, S], bf16, tag="qg_d")
        kh_d = dp.tile([512, S], bf16, tag="kh_d")
        vc_d = dp.tile([512, S], bf16, tag="vc_d")
        core_d = dp.tile([512, S], bf16, tag="core_d")

        # constants
        masks = cst.tile([128, 384], fp32, tag="masks")
        nc.sync.dma_start(out=masks, in_=masks_d.ap())
        maskLS, maskUS, maskUI = masks[:, 0:128], masks[:, 128:256], masks[:, 256:384]
        identb = cst.tile([128, 128], bf16, tag="identb")
        nc.sync.dma_start(out=identb, in_=identb_d.ap())
        identf = cst.tile([128, 128], fp32, tag="identf")
        nc.sync.dma_start(out=identf, in_=identf_d.ap())
        normw = cst.tile([128, 1], fp32, tag="normw")
        nc.sync.dma_start(out=normw, in_=normw_d.ap())
        onesb = cst.tile([128, 1], bf16, tag="onesb")
        nc.vector.memset(onesb, 1.0)
        colst = cst.tile([128, HPC * 192], fp32, tag="colst")
        nc.sync.dma_start(out=colst, in_=colst_d.ap())
        hstat = {}
        for h in range(HPC):
            o = h * 192
            hstat[h] = dict(
                gcT=colst[:, o:o + 32], bT=colst[:, o + 32:o + 64],
                nbT=colst[:, o + 64:o + 96], grevT=colst[:, o + 96:o + 128],
                bgT=colst[:, o + 128:o + 160], gtotT=colst[:, o + 160:o + 192])

        # stage sharded inputs into internal DRAM and gather on-device
        with tc.tile_pool(name="stg", bufs=2) as stg:
            t = stg.tile([128, KT, S // 4], bf16, tag="sg")
            nc.sync.dma_start(
                out=t, in_=hsQ.ap().rearrange("(kt p) t -> p kt t", p=128))
            nc.sync.dma_start(
                out=hsb.ap().rearrange("(kt p) t -> p kt t", p=128), in_=t)
            t = stg.tile([128, KT // 2, COLS], bf16, tag="sw")
            nc.sync.dma_start(
                out=t, in_=W_half.ap().rearrange("(kt p) c -> p kt c", p=128))
            nc.sync.dma_start(
                out=wb.ap().rearrange("(kt p) c -> p kt c", p=128), in_=t)
            t = stg.tile([128, 2, D], bf16, tag="so")
            nc.sync.dma_start(
                out=t, in_=Wo_half.ap().rearrange("(j p) d -> p j d", p=128))
            nc.sync.dma_start(
                out=wob.ap().rearrange("(j p) d -> p j d", p=128), in_=t)
        nc.gpsimd.collective_compute(
            "AllGather", mybir.AluOpType.bypass, replica_groups=G4,
            ins=[hsb.ap()], outs=[hsg.ap()])
        nc.gpsimd.collective_compute(
            "AllGather", mybir.AluOpType.bypass, replica_groups=G2,
            ins=[wb.ap()], outs=[wg.ap()])
        nc.gpsimd.collective_compute(
            "AllGather", mybir.AluOpType.bypass, replica_groups=G2,
            ins=[wob.ap()], outs=[wog.ap()])

        # ---------------- Phase 1: projections ----------------
        with tc.tile_pool(name="p1w", bufs=1) as wp, \
             tc.tile_pool(name="p1", bufs=3) as p1, \
             tc.tile_pool(name="p1h", bufs=2) as p1h, \
             tc.tile_pool(name="pp1", bufs=4, space="PSUM") as pp1:
            walls = wp.tile([128, KT, COLS], bf16, tag="walls")
            nc.sync.dma_start(
                out=walls, in_=wg.ap().rearrange("(kt p) c -> p kt c", p=128))
            for nt in range(NT):
                ts = slice(nt * 512, (nt + 1) * 512)
                q4 = nt // 2
                lo = (nt % 2) * 512
                hsblk = p1h.tile([128, KT, 512], bf16, tag="hsblk")
                nc.sync.dma_start(
                    out=hsblk,
                    in_=hsg.ap()[q4 * D:(q4 + 1) * D, :].rearrange(
                        "(kt p) t -> p kt t", p=128)[:, :, lo:lo + 512])
                for ct in range(NCT):
                    c0 = ct * 128
                    ps = pp1.tile([128, 512], fp32, tag="ps")
                    for k in range(KT):
                        nc.tensor.matmul(
                            ps, walls[:, k, c0:c0 + 128], hsblk[:, k, :],
                            start=(k == 0), stop=(k == KT - 1))
                    t = p1.tile([128, 512], bf16, tag="t")
                    if ct < 12:
                        nc.vector.tensor_copy(out=t, in_=ps)
                        nc.sync.dma_start(out=mixed_d[c0:c0 + 128, ts], in_=t)
                    else:
                        nc.scalar.activation(out=t, in_=ps, func=AF.Silu)
                        nc.sync.dma_start(
                            out=z_d[(ct - CT_Z) * 128:(ct - CT_Z) * 128 + 128, ts],
                            in_=t)

        # ---------------- Phase 2b: conv + silu + l2norm ----------------
        epsq = cst.tile([1, 1], fp32, tag="epsq")
        nc.vector.memset(epsq, EPS * 128.0)
        epsk = cst.tile([1, 1], fp32, tag="epsk")
        nc.vector.memset(epsk, EPS)
        epsO = cst.tile([128, 1], fp32, tag="epsO")
        nc.vector.memset(epsO, EPS)

        with tc.tile_pool(name="p2b", bufs=2) as p2, \
             tc.tile_pool(name="p2r", bufs=2) as p2r, \
             tc.tile_pool(name="pp2", bufs=8, space="PSUM") as pp2:
            for ct in range(12):
                c0 = ct * 128
                pl = p2.tile([128, S], bf16, tag="pl")
                nc.sync.dma_start(out=pl, in_=mixed_d[c0:c0 + 128, :])
                cwt = p2r.tile([128, KCONV], fp32, tag="cwt")
                nc.sync.dma_start(out=cwt, in_=convw.ap()[c0:c0 + 128, :])
                acc = p2.tile([128, S], bf16, tag="acc")
                nc.vector.tensor_scalar_mul(out=acc, in0=pl, scalar1=cwt[:, 3:4])
                for j in range(3):
                    sh = 3 - j
                    nc.vector.scalar_tensor_tensor(
                        out=acc[:, sh:], in0=pl[:, :S - sh], scalar=cwt[:, j:j + 1],
                        in1=acc[:, sh:], op0=mybir.AluOpType.mult,
                        op1=mybir.AluOpType.add)
                sil = p2.tile([128, S], bf16, tag="sil")
                nc.scalar.activation(out=sil, in_=acc, func=AF.Silu)
                if ct < 8:  # q or k: l2 normalize over dk (partition dim)
                    isq = ct < 4
                    h = ct if isq else ct - 4
                    sq = p2.tile([128, S], bf16, tag="sq")
                    nc.scalar.activation(out=sq, in_=sil, func=AF.Square)
                    srow = p2r.tile([1, S], fp32, tag="srow")
                    for nt in range(NT):
                        ts = slice(nt * 512, (nt + 1) * 512)
                        pss = pp2.tile([1, 512], fp32, tag="pss")
                        nc.tensor.matmul(pss, onesb, sq[:, ts], start=True, stop=True)
                        nc.scalar.activation(
                            out=srow[:, ts], in_=pss, func=AF.Sqrt,
                            bias=epsq if isq else epsk,
                            scale=128.0 if isq else 1.0)
                    nc.vector.reciprocal(out=srow, in_=srow)
                    rrow2 = p2r.tile([1, S], bf16, tag="rrow2")
                    nc.vector.tensor_copy(out=rrow2, in_=srow)
                    brd = p2.tile([128, S], bf16, tag="brd")
                    nc.gpsimd.partition_broadcast(brd, rrow2)
                    opl = p2.tile([128, S], bf16, tag="opl")
                    nc.vector.tensor_mul(out=opl, in0=sil, in1=brd)
                    dst = qg_d if isq else kh_d
                    nc.sync.dma_start(out=dst[h * 128:h * 128 + 128, :], in_=opl)
                    if debug:
                        dd = dbg["d_qg"] if isq else dbg["d_kh"]
                        nc.sync.dma_start(out=dd.ap()[h * 128:h * 128 + 128, :],
                                          in_=opl)
                else:
                    nc.sync.dma_start(
                        out=vc_d[(ct - 8) * 128:(ct - 8) * 128 + 128, :], in_=sil)
                    if debug:
                        nc.sync.dma_start(
                            out=dbg["d_vc"].ap()[(ct - 8) * 128:(ct - 8) * 128 + 128, :],
                            in_=sil)
        if debug:
            with tc.tile_pool(name="dbgp", bufs=2) as dbp:
                for ct in range(12):
                    t = dbp.tile([128, S], bf16, tag="t")
                    nc.sync.dma_start(out=t, in_=mixed_d[ct * 128:(ct + 1) * 128, :])
                    nc.sync.dma_start(out=dbg["d_mixed"].ap()[ct * 128:(ct + 1) * 128, :], in_=t)
                for ct in range(4):
                    t = dbp.tile([128, S], bf16, tag="t")
                    nc.sync.dma_start(out=t, in_=z_d[ct * 128:(ct + 1) * 128, :])
                    nc.sync.dma_start(out=dbg["d_z"].ap()[ct * 128:(ct + 1) * 128, :], in_=t)

        # ---------------- Phase 3: chunked gated delta rule ----------------
        with tc.tile_pool(name="p3pl", bufs=2) as p3p, \
             tc.tile_pool(name="p3row", bufs=1) as p3r, \
             tc.tile_pool(name="p3brd", bufs=1) as p3b, \
             tc.tile_pool(name="p3w", bufs=3) as p3, \
             tc.tile_pool(name="p3sq", bufs=4) as p3s, \
             tc.tile_pool(name="p3st", bufs=3) as p3st, \
             tc.tile_pool(name="ppA", bufs=1, space="PSUM") as ppA, \
             tc.tile_pool(name="ppB", bufs=1, space="PSUM") as ppB, \
             tc.tile_pool(name="ppT", bufs=1, space="PSUM") as ppT, \
             tc.tile_pool(name="ppX", bufs=1, space="PSUM") as ppX, \
             tc.tile_pool(name="ppS", bufs=1, space="PSUM") as ppS:
            for h in range(HPC):
                st = hstat[h]
                qp = p3p.tile([128, S], bf16, tag="qp")
                nc.sync.dma_start(out=qp, in_=qg_d[h * 128:h * 128 + 128, :])
                kp = p3p.tile([128, S], bf16, tag="kp")
                nc.sync.dma_start(out=kp, in_=kh_d[h * 128:h * 128 + 128, :])
                vp = p3p.tile([128, S], bf16, tag="vp")
                nc.sync.dma_start(out=vp, in_=vc_d[h * 128:h * 128 + 128, :])
                szp = p3p.tile([128, S], bf16, tag="szp")
                nc.sync.dma_start(out=szp, in_=z_d[h * 128:h * 128 + 128, :])
                cp = p3p.tile([128, S], bf16, tag="cp")
                gcrow = p3r.tile([1, S], fp32, tag="gcrow")
                nc.sync.dma_start(out=gcrow, in_=growm_d.ap()[h:h + 1, :])
                ngcrow = p3r.tile([1, S], fp32, tag="ngcrow")
                nc.sync.dma_start(out=ngcrow, in_=growm_d.ap()[HPC + h:HPC + h + 1, :])
                gbrdF = p3b.tile([128, S], fp32, tag="gbrdF")
                nc.gpsimd.partition_broadcast(gbrdF, gcrow)
                ngbrdF = p3b.tile([128, S], fp32, tag="ngbrdF")
                nc.gpsimd.partition_broadcast(ngbrdF, ngcrow)
                Scur = p3st.tile([128, 128], bf16, tag="S")
                nc.vector.memset(Scur, 0.0)
                for c in range(NCHUNK):
                    cs = slice(c * C, (c + 1) * C)
                    col = slice(c, c + 1)
                    gbrd = gbrdF[:, cs]
                    ngbrd = ngbrdF[:, cs]
                    # decay matrices
                    dS = p3.tile([128, 128], fp32, tag="dS")
                    nc.vector.scalar_tensor_tensor(
                        out=dS, in0=ngbrd, scalar=st["gcT"][:, col], in1=maskLS,
                        op0=OP.add, op1=OP.add)
                    nc.scalar.activation(out=dS, in_=dS, func=AF.Exp)
                    dST = p3.tile([128, 128], fp32, tag="dST")
                    nc.vector.scalar_tensor_tensor(
                        out=dST, in0=gbrd, scalar=st["gcT"][:, col], in1=maskUS,
                        op0=OP.subtract, op1=OP.add)
                    nc.scalar.activation(out=dST, in_=dST, func=AF.Exp)
                    dIT = p3.tile([128, 128], fp32, tag="dIT")
                    nc.vector.scalar_tensor_tensor(
                        out=dIT, in0=gbrd, scalar=st["gcT"][:, col], in1=maskUI,
                        op0=OP.subtract, op1=OP.add)
                    nc.scalar.activation(out=dIT, in_=dIT, func=AF.Exp)
                    # KK^T and KQ^T
                    psA = ppA.tile([128, 256], fp32, tag="psA")
                    nc.tensor.matmul(psA[:, 0:128], kp[:, cs], kp[:, cs],
                                     start=True, stop=True)
                    nc.tensor.matmul(psA[:, 128:256], kp[:, cs], qp[:, cs],
                                     start=True, stop=True)
                    B0 = p3.tile([128, 128], bf16, tag="B0")
                    nc.vector.scalar_tensor_tensor(
                        out=B0, in0=psA[:, 0:128], scalar=st["nbT"][:, col],
                        in1=dS, op0=OP.mult, op1=OP.mult)
                    # transposes: B^T, K_tm, V_tm
                    psT = ppT.tile([128, 384], bf16, tag="psT")
                    nc.tensor.transpose(psT[:, 0:128], B0, identb)
                    BT0 = p3.tile([128, 128], bf16, tag="BT0")
                    nc.vector.tensor_copy(out=BT0, in_=psT[:, 0:128])
                    nc.tensor.transpose(psT[:, 128:256], kp[:, cs], identb)
                    Ktm = p3.tile([128, 128], bf16, tag="Ktm")
                    nc.vector.tensor_copy(out=Ktm, in_=psT[:, 128:256])
                    nc.tensor.transpose(psT[:, 256:384], vp[:, cs], identb)
                    X = p3s.tile([128, 256], bf16, tag="X")
                    nc.vector.tensor_scalar_mul(
                        out=X[:, 0:128], in0=Ktm, scalar1=st["bgT"][:, col])
                    nc.vector.tensor_scalar_mul(
                        out=X[:, 128:256], in0=psT[:, 256:384],
                        scalar1=st["bT"][:, col])
                    Kpr = p3.tile([128, 128], bf16, tag="Kpr")
                    nc.vector.tensor_scalar_mul(
                        out=Kpr, in0=Ktm, scalar1=st["grevT"][:, col])
                    # Neumann product chain
                    Sq, SqT = B0, BT0
                    for j in range(7):
                        psX = ppX.tile([128, 256], fp32, tag="psX")
                        nc.tensor.matmul(psX, SqT, X, start=True, stop=True)
                        Xn = p3s.tile([128, 256], bf16, tag="X")
                        nc.vector.tensor_add(out=Xn, in0=X, in1=psX)
                        X = Xn
                        if j < 6:
                            psq = ppX.tile([128, 256], fp32, tag="psq")
                            nSqT = p3s.tile([128, 128], bf16, tag="nSqT")
                            nc.tensor.matmul(psq[:, 128:256], Sq, SqT,
                                             start=True, stop=True)
                            nc.vector.tensor_copy(out=nSqT, in_=psq[:, 128:256])
                            if j < 5:
                                nSq = p3s.tile([128, 128], bf16, tag="nSq")
                                nc.tensor.matmul(psq[:, 0:128], SqT, Sq,
                                                 start=True, stop=True)
                                nc.vector.tensor_copy(out=nSq, in_=psq[:, 0:128])
                                Sq = nSq
                            SqT = nSqT
                    # P^T, W^T P^T, H^T
                    PT = p3.tile([128, 128], bf16, tag="PT")
                    nc.vector.tensor_mul(out=PT, in0=psA[:, 128:256], in1=dIT)
                    psB = ppB.tile([128, 256], fp32, tag="psB")
                    nc.tensor.matmul(psB[:, 0:128], X[:, 0:128], PT,
                                     start=True, stop=True)
                    gamB = p3.tile([128, 128], fp32, tag="gamB")
                    nc.scalar.activation(out=gamB, in_=gbrd, func=AF.Exp)
                    QtG = p3.tile([128, 128], bf16, tag="QtG")
                    nc.vector.tensor_mul(out=QtG, in0=qp[:, cs], in1=gamB)
                    QtT = p3.tile([128, 128], bf16, tag="QtT")
                    nc.vector.tensor_sub(out=QtT, in0=QtG, in1=psB[:, 0:128])
                    nc.tensor.matmul(psB[:, 128:256], X[:, 0:128], Kpr,
                                     start=True, stop=True)
                    GT = p3.tile([128, 128], bf16, tag="GT")
                    nc.vector.scalar_tensor_tensor(
                        out=GT, in0=identf, scalar=st["gtotT"][:, col],
                        in1=psB[:, 128:256], op0=OP.mult, op1=OP.subtract)
                    # O and state update
                    psS = ppS.tile([128, 256], fp32, tag="psS")
                    nc.tensor.matmul(psS[:, 0:128], PT, X[:, 128:256],
                                     start=True, stop=False)
                    nc.tensor.matmul(psS[:, 0:128], QtT, Scur,
                                     start=False, stop=True)
                    nc.tensor.matmul(psS[:, 128:256], GT, Scur,
                                     start=True, stop=False)
                    nc.tensor.matmul(psS[:, 128:256], Kpr, X[:, 128:256],
                                     start=False, stop=True)
                    Snew = p3st.tile([128, 128], bf16, tag="S")
                    nc.vector.tensor_copy(out=Snew, in_=psS[:, 128:256])
                    # gated RMS norm of O, transpose back, gate with silu(z)
                    osq = p3.tile([128, 128], bf16, tag="osq")
                    ssum = p3.tile([128, 1], fp32, tag="ssum")
                    nc.scalar.activation(out=osq, in_=psS[:, 0:128], func=AF.Square,
                                         accum_out=ssum)
                    sr = p3.tile([128, 1], fp32, tag="sr")
                    nc.scalar.activation(out=sr, in_=ssum, func=AF.Sqrt,
                                         bias=epsO, scale=1.0 / 128.0)
                    rr = p3.tile([128, 1], fp32, tag="rr")
                    nc.vector.reciprocal(out=rr, in_=sr)
                    otm = p3.tile([128, 128], bf16, tag="otm")
                    nc.vector.tensor_scalar_mul(out=otm, in0=psS[:, 0:128],
                                                scalar1=rr)
                    psO = ppT.tile([128, 128], bf16, tag="psO")
                    nc.tensor.transpose(psO, otm, identb)
                    nc.vector.scalar_tensor_tensor(
                        out=cp[:, cs], in0=psO, scalar=normw, in1=szp[:, cs],
                        op0=OP.mult, op1=OP.mult)
                    Scur = Snew
                nc.sync.dma_start(out=core_d[h * 128:h * 128 + 128, :], in_=cp)
                if debug:
                    nc.sync.dma_start(
                        out=dbg["d_core"].ap()[h * 128:h * 128 + 128, :], in_=cp)

        # ---------------- Phase 4: out projection ----------------
        with tc.tile_pool(name="p4w", bufs=1) as p4w, \
             tc.tile_pool(name="p4", bufs=3) as p4, \
             tc.tile_pool(name="pp4", bufs=4, space="PSUM") as pp4:
            wot = p4w.tile([128, HPC, D], bf16, tag="wot")
            nc.sync.dma_start(
                out=wot, in_=wog.ap().rearrange("(j p) d -> p j d", p=128))
            for nt in range(NT):
                ts = slice(nt * 512, (nt + 1) * 512)
                cblk = p4.tile([128, HPC, 512], bf16, tag="cblk")
                nc.sync.dma_start(
                    out=cblk,
                    in_=core_d.rearrange("(j p) t -> p j t", p=128)[:, :, ts])
                for dt in range(16):
                    ps = pp4.tile([128, 512], fp32, tag="ps")
                    for j in range(HPC):
                        nc.tensor.matmul(ps, wot[:, j, dt * 128:(dt + 1) * 128],
                                         cblk[:, j, :],
                                         start=(j == 0), stop=(j == HPC - 1))
                    ot = p4.tile([128, 512], bf16, tag="ot")
                    nc.vector.tensor_copy(out=ot, in_=ps)
                    nc.sync.dma_start(out=rs_in.ap()[dt * 128:(dt + 1) * 128, ts],
                                      in_=ot)

        nc.gpsimd.collective_compute(
            "ReduceScatter", mybir.AluOpType.add, replica_groups=G4,
            ins=[rs_in.ap()], outs=[rs_out.ap()])
        with tc.tile_pool(name="fin", bufs=2) as fin:
            for j in range(4):
                t = fin.tile([128, S], bf16, tag="fo")
                nc.sync.dma_start(out=t, in_=rs_out.ap()[j * 128:(j + 1) * 128, :])
                nc.sync.dma_start(out=outR.ap()[j * 128:(j + 1) * 128, :], in_=t)

    nc.compile()
    return nc


def _host_inputs(hidden_states, W_qkv, W_z, W_b, W_a, conv_w, A_log, dt_bias,
                 norm_w, W_out):
    """Per-core input maps.  beta/g decay stats computed on host (tiny)."""
    masks = np.zeros((128, 384), np.float32)
    r = np.arange(128)
    masks[:, 0:128] = np.where(r[None, :] < r[:, None], 0.0, NEG)     # s<t
    masks[:, 128:256] = np.where(r[None, :] > r[:, None], 0.0, NEG)   # f>p
    masks[:, 256:384] = np.where(r[None, :] >= r[:, None], 0.0, NEG)  # f>=p
    identity = np.eye(128, dtype=np.float32)

    hs2 = hidden_states.reshape(B * S, D).astype(np.float32)
    bet = 1.0 / (1.0 + np.exp(-(hs2 @ W_b)))                    # [B*S, 16]
    g = (-np.exp(A_log.astype(np.float32))[None, :]
         * np.logaddexp(0.0, hs2 @ W_a + dt_bias[None, :]))     # [B*S, 16]
    bet = bet.reshape(B, NCHUNK, C, HK)
    g = g.reshape(B, NCHUNK, C, HK)
    gc_all = np.cumsum(g, axis=2)                               # [B, nc, C, H]

    in_maps = []
    for c in range(8):
        b = c // 4
        h0 = 4 * (c % 4)
        ch = slice(h0 * 128, (h0 + 4) * 128)
        W_all = np.concatenate([
            W_qkv[:, h0 * 128:(h0 + 4) * 128],
            W_qkv[:, KEY_DIM + h0 * 128:KEY_DIM + (h0 + 4) * 128],
            W_qkv[:, 2 * KEY_DIM + h0 * 128:2 * KEY_DIM + (h0 + 4) * 128],
            W_z[:, h0 * 128:(h0 + 4) * 128],
        ], axis=1).astype(BF16)
        cw = np.concatenate([
            conv_w[h0 * 128:(h0 + 4) * 128, 0, :],
            conv_w[KEY_DIM + h0 * 128:KEY_DIM + (h0 + 4) * 128, 0, :],
            conv_w[2 * KEY_DIM + h0 * 128:2 * KEY_DIM + (h0 + 4) * 128, 0, :],
        ], axis=0).astype(np.float32)

        growm = np.zeros((2 * HPC, S), np.float32)
        colst = np.zeros((128, HPC * 192), np.float32)
        for j in range(HPC):
            gc = gc_all[b, :, :, h0 + j]                        # [nc, C]
            be = bet[b, :, :, h0 + j]
            gam = np.exp(gc)
            growm[j] = gc.reshape(S)
            growm[HPC + j] = -gc.reshape(S)
            o = j * 192
            colst[:, o:o + 32] = gc.T                           # gcT
            colst[:, o + 32:o + 64] = be.T                      # bT
            colst[:, o + 64:o + 96] = -be.T                     # nbT
            colst[:, o + 96:o + 128] = np.exp(gc[:, -1][None, :] - gc.T)  # grevT
            colst[:, o + 128:o + 160] = (be * gam).T            # bgT
            colst[:, o + 160:o + 192] = np.exp(gc[:, -1])[None, :] * np.ones((128, 1), np.float32)  # gtotT
        q4 = c % 4
        half = slice(0, D // 2) if b == 0 else slice(D // 2, D)
        oh = slice(h0 * 128, h0 * 128 + 256) if b == 0 else \
             slice(h0 * 128 + 256, (h0 + 4) * 128)
        in_maps.append({
            "hsQ": np.ascontiguousarray(
                hidden_states[b, q4 * (S // 4):(q4 + 1) * (S // 4), :].T
                ).astype(BF16),
            "W_half": np.ascontiguousarray(W_all[half, :]),
            "Wo_half": W_out[oh, :].astype(BF16),
            "convw": np.ascontiguousarray(cw),
            "growm": growm,
            "colstats": colst,
            "normw": norm_w.astype(np.float32).reshape(128, 1),
            "masks": masks,
            "identb": identity.astype(BF16),
            "identf": identity,
        })
    return in_maps


def _setup_jax_cache():
    try:
        import jax
        cache_dir = "/var/tmp/jaxcache"
        os.makedirs(cache_dir, exist_ok=True)
        jax.config.update("jax_compilation_cache_dir", cache_dir)
        jax.config.update("jax_persistent_cache_min_entry_size_bytes", 0)
        jax.config.update("jax_persistent_cache_min_compile_time_secs", 0.0)
    except Exception:
        pass


def kernel(hidden_states, W_qkv, W_z, W_b, W_a, conv_w, A_log, dt_bias,
           norm_w, W_out):
    from concourse import bass_utils

    _setup_jax_cache()
    if "nc" not in _CACHE:
        _CACHE["nc"] = build_nc(debug=False)
    nc = _CACHE["nc"]
    in_maps = _host_inputs(hidden_states, W_qkv, W_z, W_b, W_a, conv_w,
                           A_log, dt_bias, norm_w, W_out)
    res = bass_utils.run_bass_kernel_spmd(nc, in_maps, core_ids=list(range(8)))
    out = np.empty((B, S, D), np.float32)
    for b in range(B):
        outT = np.concatenate(
            [res.results[b * 4 + r]["outR"] for r in range(4)], axis=0)
        out[b] = outT.astype(np.float32).T
    return out


# revision 33
# speedup vs baseline: 68.7872x; 3.4124x over previous
"""GatedDeltaNet Trainium2 kernel (8 NeuronCores, SPMD).

Shapes: B=2, S=4096, D=2048, HK=HV=16, DK=DV=128, KCONV=4.
Sharding: core c -> batch b=c//4, heads h0=4*(c%4) .. h0+4 (batch x head
parallel).  Each core computes its 4 heads' full pipeline and a partial
out-projection [D, S]; the host sums 4 partials per batch and transposes.

Device algorithm (per core), all activations channel-major [chan, time]:
  P1  fused projection: mixed/z/beta/alpha = W_all^T @ hs^T  (bf16 matmuls)
  P2  depthwise causal conv (4 taps) + SiLU; l2-norm of q,k (partition-dim
      sums via ones-matmul); per-head decay stats (cumsum via DVE scan)
  P3  chunked gated delta rule, chunk C=128:
        M[t,s] = beta_t (k_t.k_s) exp(gc_t-gc_s) (s<t);  B = -M
        [W|U] = (I+M)^{-1} [beta*Gam*K | beta*V] via product
                 prod_j (I + B^{2^j}) (B nilpotent, 7 levels, all matmuls)
        O = P@U + (Gam*Q - P@W) @ S0,  P[t,s] = (q_t.k_s) exp(gc_t-gc_s) (s<=t)
        S' = (gtot*I - K'^T W) @ S0 + K'^T U,  K'_t = exp(gc_C-gc_t) k_t
      then gated RMSNorm and gate with silu(z)
  P4  row-parallel out-projection partial: out^T += W_out_slice^T @ core^T
"""

import os
import numpy as np
import ml_dtypes

B, S, D = 2, 4096, 2048
HK, HV, DK, DV, KCONV = 16, 16, 128, 128, 4
KEY_DIM, VALUE_DIM = HK * DK, HV * DV
CONV_DIM = 2 * KEY_DIM + VALUE_DIM
EPS = 1e-6
HPC = 4            # heads per core
C = 128            # chunk length
NCHUNK = S // C    # 32
NT = S // 512      # 8 token blocks
KT = D // 128      # 16 k-tiles
CT_Q, CT_K, CT_V, CT_Z = 0, 4, 8, 12   # ctile index bases
NCT = 16           # 16 x 128 cols (q|k|v|z)
COLS = 2048

BF16 = ml_dtypes.bfloat16
NEG = -1e30

_CACHE = {}


def build_nc(debug=False):
    import concourse.bass as bass
    import concourse.mybir as mybir
    import concourse.tile as tile
    from concourse import bacc

    fp32 = mybir.dt.float32
    bf16 = mybir.dt.bfloat16
    AF = mybir.ActivationFunctionType
    OP = mybir.AluOpType
    AX = mybir.AxisListType

    nc = bacc.Bacc("TRN2", target_bir_lowering=False, debug=False, num_devices=8)

    G4 = [[0, 1, 2, 3], [4, 5, 6, 7]]       # batch groups (token AG, out RS)
    G2 = [[0, 4], [1, 5], [2, 6], [3, 7]]   # head-group pairs (weight AG)

    hsQ = nc.dram_tensor("hsQ", [D, S // 4], bf16, kind="ExternalInput")
    W_half = nc.dram_tensor("W_half", [D // 2, COLS], bf16, kind="ExternalInput")
    Wo_half = nc.dram_tensor("Wo_half", [HPC * DV // 2, D], bf16,
                             kind="ExternalInput")
    hsb = nc.dram_tensor("hsb", [D, S // 4], bf16)
    hsg = nc.dram_tensor("hsg", [4 * D, S // 4], bf16)
    wb = nc.dram_tensor("wb", [D // 2, COLS], bf16)
    wg = nc.dram_tensor("wg", [D, COLS], bf16)
    wob = nc.dram_tensor("wob", [HPC * DV // 2, D], bf16)
    wog = nc.dram_tensor("wog", [HPC * DV, D], bf16)
    rs_in = nc.dram_tensor("rs_in", [D, S], bf16)
    rs_out = nc.dram_tensor("rs_out", [D // 4, S], bf16)
    convw = nc.dram_tensor("convw", [12 * 128, KCONV], fp32, kind="ExternalInput")
    growm_d = nc.dram_tensor("growm", [2 * HPC, S], fp32, kind="ExternalInput")
    colst_d = nc.dram_tensor("colstats", [128, HPC * 192], fp32, kind="ExternalInput")
    normw_d = nc.dram_tensor("normw", [128, 1], fp32, kind="ExternalInput")
    masks_d = nc.dram_tensor("masks", [128, 384], fp32, kind="ExternalInput")
    identb_d = nc.dram_tensor("identb", [128, 128], bf16, kind="ExternalInput")
    identf_d = nc.dram_tensor("identf", [128, 128], fp32, kind="ExternalInput")
    outR = nc.dram_tensor("outR", [D // 4, S], bf16, kind="ExternalOutput")

    dbg = {}
    if debug:
        for nm, shp in [("d_mixed", [1536, S]), ("d_qg", [512, S]),
                        ("d_kh", [512, S]), ("d_vc", [512, S]),
                        ("d_z", [512, S]), ("d_core", [512, S])]:
            dbg[nm] = nc.dram_tensor(nm, shp, bf16, kind="ExternalOutput")

    from contextlib import ExitStack
    with tile.TileContext(nc) as tc, ExitStack() as ctx:
        cst = ctx.enter_context(tc.tile_pool(name="cst", bufs=1))
        dp = ctx.enter_context(tc.tile_pool(name="dram", bufs=1, space="DRAM"))

        mixed_d = dp.tile([1536, S], bf16, tag="mixed_d")
        z_d = dp.tile([512, S], bf16, tag="z_d")
        qg_d = dp.tile([512, S], bf16, tag="qg_d")
        kh_d = dp.tile([512, S], bf16, tag="kh_d")
        vc_d = dp.tile([512, S], bf16, tag="vc_d")
        core_d = dp.tile([512, S], bf16, tag="core_d")

        # constants
        masks = cst.tile([128, 384], fp32, tag="masks")
        nc.sync.dma_start(out=masks, in_=masks_d.ap())
        maskLS, maskUS, maskUI = masks[:, 0:128], masks[:, 128:256], masks[:, 256:384]
        identb = cst.tile([128, 128], bf16, tag="identb")
        nc.sync.dma_start(out=identb, in_=identb_d.ap())
        identf = cst.tile([128, 128], fp32, tag="identf")
        nc.sync.dma_start(out=identf, in_=identf_d.ap())
        normw = cst.tile([128, 1], fp32, tag="normw")
        nc.sync.dma_start(out=normw, in_=normw_d.ap())
        onesb = cst.tile([128, 1], bf16, tag="onesb")
        nc.vector.memset(onesb, 1.0)
        colst = cst.tile([128, HPC * 192], fp32, tag="colst")
        nc.sync.dma_start(out=colst, in_=colst_d.ap())
        hstat = {}
        for h in range(HPC):
            o = h * 192
            hstat[h] = dict(
                gcT=colst[:, o:o + 32], bT=colst[:, o + 32:o + 64],
                nbT=colst[:, o + 64:o + 96], grevT=colst[:, o + 96:o + 128],
                bgT=colst[:, o + 128:o + 160], gtotT=colst[:, o + 160:o + 192])

        # stage sharded inputs into internal DRAM and gather on-device
        with tc.tile_pool(name="stg", bufs=2) as stg:
            t = stg.tile([128, KT, S // 4], bf16, tag="sg")
            nc.sync.dma_start(
                out=t, in_=hsQ.ap().rearrange("(kt p) t -> p kt t", p=128))
            nc.sync.dma_start(
                out=hsb.ap().rearrange("(kt p) t -> p kt t", p=128), in_=t)
            t = stg.tile([128, KT // 2, COLS], bf16, tag="sw")
            nc.sync.dma_start(
                out=t, in_=W_half.ap().rearrange("(kt p) c -> p kt c", p=128))
            nc.sync.dma_start(
                out=wb.ap().rearrange("(kt p) c -> p kt c", p=128), in_=t)
            t = stg.tile([128, 2, D], bf16, tag="so")
            nc.sync.dma_start(
                out=t, in_=Wo_half.ap().rearrange("(j p) d -> p j d", p=128))
            nc.sync.dma_start(
                out=wob.ap().rearrange("(j p) d -> p j d", p=128), in_=t)
        nc.gpsimd.collective_compute(
            "AllGather", mybir.AluOpType.bypass, replica_groups=G4,
            ins=[hsb.ap()], outs=[hsg.ap()])
        nc.gpsimd.collective_compute(
            "AllGather", mybir.AluOpType.bypass, replica_groups=G2,
            ins=[wb.ap()], outs=[wg.ap()])
        nc.gpsimd.collective_compute(
            "AllGather", mybir.AluOpType.bypass, replica_groups=G2,
            ins=[wob.ap()], outs=[wog.ap()])

        # ---------------- Phase 1: projections ----------------
        with tc.tile_pool(name="p1w", bufs=1) as wp, \
             tc.tile_pool(name="p1", bufs=3) as p1, \
             tc.tile_pool(name="p1h", bufs=2) as p1h, \
             tc.tile_pool(name="pp1", bufs=4, space="PSUM") as pp1:
            walls = wp.tile([128, KT, COLS], bf16, tag="walls")
            nc.sync.dma_start(
                out=walls, in_=wg.ap().rearrange("(kt p) c -> p kt c", p=128))
            for nt in range(NT):
                ts = slice(nt * 512, (nt + 1) * 512)
                q4 = nt // 2
                lo = (nt % 2) * 512
                hsblk = p1h.tile([128, KT, 512], bf16, tag="hsblk")
                nc.sync.dma_start(
                    out=hsblk,
                    in_=hsg.ap()[q4 * D:(q4 + 1) * D, :].rearrange(
                        "(kt p) t -> p kt t", p=128)[:, :, lo:lo + 512])
                for ct in range(NCT):
                    c0 = ct * 128
                    ps = pp1.tile([128, 512], fp32, tag="ps")
                    for k in range(KT):
                        nc.tensor.matmul(
                            ps, walls[:, k, c0:c0 + 128], hsblk[:, k, :],
                            start=(k == 0), stop=(k == KT - 1))
                    t = p1.tile([128, 512], bf16, tag="t")
                    if ct < 12:
                        nc.scalar.activation(out=t, in_=ps, func=AF.Copy)
                        nc.sync.dma_start(out=mixed_d[c0:c0 + 128, ts], in_=t)
                    else:
                        nc.scalar.activation(out=t, in_=ps, func=AF.Silu)
                        nc.sync.dma_start(
                            out=z_d[(ct - CT_Z) * 128:(ct - CT_Z) * 128 + 128, ts],
                            in_=t)

        # ---------------- Phase 2b: conv + silu + l2norm ----------------
        epsq = cst.tile([1, 1], fp32, tag="epsq")
        nc.vector.memset(epsq, EPS * 128.0)
        epsk = cst.tile([1, 1], fp32, tag="epsk")
        nc.vector.memset(epsk, EPS)
        epsO = cst.tile([128, 1], fp32, tag="epsO")
        nc.vector.memset(epsO, EPS)

        with tc.tile_pool(name="p2b", bufs=2) as p2, \
             tc.tile_pool(name="p2r", bufs=2) as p2r, \
             tc.tile_pool(name="pp2", bufs=8, space="PSUM") as pp2:
            for ct in range(12):
                c0 = ct * 128
                pl = p2.tile([128, S], bf16, tag="pl")
                nc.sync.dma_start(out=pl, in_=mixed_d[c0:c0 + 128, :])
                cwt = p2r.tile([128, KCONV], fp32, tag="cwt")
                nc.sync.dma_start(out=cwt, in_=convw.ap()[c0:c0 + 128, :])
                acc = p2.tile([128, S], bf16, tag="acc")
                nc.vector.tensor_scalar_mul(out=acc, in0=pl, scalar1=cwt[:, 3:4])
                for j in range(3):
                    sh = 3 - j
                    nc.vector.scalar_tensor_tensor(
                        out=acc[:, sh:], in0=pl[:, :S - sh], scalar=cwt[:, j:j + 1],
                        in1=acc[:, sh:], op0=mybir.AluOpType.mult,
                        op1=mybir.AluOpType.add)
                sil = p2.tile([128, S], bf16, tag="sil")
                nc.scalar.activation(out=sil, in_=acc, func=AF.Silu)
                if ct < 8:  # q or k: l2 normalize over dk (partition dim)
                    isq = ct < 4
                    h = ct if isq else ct - 4
                    sq = p2.tile([128, S], bf16, tag="sq")
                    nc.scalar.activation(out=sq, in_=sil, func=AF.Square)
                    srow = p2r.tile([1, S], fp32, tag="srow")
                    for nt in range(NT):
                        ts = slice(nt * 512, (nt + 1) * 512)
                        pss = pp2.tile([1, 512], fp32, tag="pss")
                        nc.tensor.matmul(pss, onesb, sq[:, ts], start=True, stop=True)
                        nc.scalar.activation(
                            out=srow[:, ts], in_=pss, func=AF.Sqrt,
                            bias=epsq if isq else epsk,
                            scale=128.0 if isq else 1.0)
                    nc.vector.reciprocal(out=srow, in_=srow)
                    rrow2 = p2r.tile([1, S], bf16, tag="rrow2")
                    nc.vector.tensor_copy(out=rrow2, in_=srow)
                    brd = p2.tile([128, S], bf16, tag="brd")
                    nc.gpsimd.partition_broadcast(brd, rrow2)
                    opl = p2.tile([128, S], bf16, tag="opl")
                    nc.vector.tensor_mul(out=opl, in0=sil, in1=brd)
                    dst = qg_d if isq else kh_d
                    nc.sync.dma_start(out=dst[h * 128:h * 128 + 128, :], in_=opl)
                    if debug:
                        dd = dbg["d_qg"] if isq else dbg["d_kh"]
                        nc.sync.dma_start(out=dd.ap()[h * 128:h * 128 + 128, :],
                                          in_=opl)
                else:
                    nc.sync.dma_start(
                        out=vc_d[(ct - 8) * 128:(ct - 8) * 128 + 128, :], in_=sil)
                    if debug:
                        nc.sync.dma_start(
                            out=dbg["d_vc"].ap()[(ct - 8) * 128:(ct - 8) * 128 + 128, :],
                            in_=sil)
        if debug:
            with tc.tile_pool(name="dbgp", bufs=2) as dbp:
                for ct in range(12):
                    t = dbp.tile([128, S], bf16, tag="t")
                    nc.sync.dma_start(out=t, in_=mixed_d[ct * 128:(ct + 1) * 128, :])
                    nc.sync.dma_start(out=dbg["d_mixed"].ap()[ct * 128:(ct + 1) * 128, :], in_=t)
                for ct in range(4):
                    t = dbp.tile([128, S], bf16, tag="t")
                    nc.sync.dma_start(out=t, in_=z_d[ct * 128:(ct + 1) * 128, :])
                    nc.sync.dma_start(out=dbg["d_z"].ap()[ct * 128:(ct + 1) * 128, :], in_=t)

        # ---------------- Phase 3: chunked gated delta rule ----------------
        with tc.tile_pool(name="p3pl", bufs=2) as p3p, \
             tc.tile_pool(name="p3pl1", bufs=1) as p3p1, \
             tc.tile_pool(name="p3row", bufs=1) as p3r, \
             tc.tile_pool(name="p3brd", bufs=1) as p3b, \
             tc.tile_pool(name="p3w", bufs=3) as p3, \
             tc.tile_pool(name="p3sq", bufs=4) as p3s, \
             tc.tile_pool(name="p3st", bufs=3) as p3st, \
             tc.tile_pool(name="ppA", bufs=1, space="PSUM") as ppA, \
             tc.tile_pool(name="ppB", bufs=1, space="PSUM") as ppB, \
             tc.tile_pool(name="ppT", bufs=1, space="PSUM") as ppT, \
             tc.tile_pool(name="ppX", bufs=2, space="PSUM") as ppX, \
             tc.tile_pool(name="ppS", bufs=1, space="PSUM") as ppS:
            for h in range(HPC):
                st = hstat[h]
                qp = p3p.tile([128, S], bf16, tag="qp")
                nc.sync.dma_start(out=qp, in_=qg_d[h * 128:h * 128 + 128, :])
                kp = p3p.tile([128, S], bf16, tag="kp")
                nc.sync.dma_start(out=kp, in_=kh_d[h * 128:h * 128 + 128, :])
                vp = p3p.tile([128, S], bf16, tag="vp")
                nc.sync.dma_start(out=vp, in_=vc_d[h * 128:h * 128 + 128, :])
                szp = p3p1.tile([128, S], bf16, tag="szp")
                nc.sync.dma_start(out=szp, in_=z_d[h * 128:h * 128 + 128, :])
                cp = p3p.tile([128, S], bf16, tag="cp")
                otmP = p3p.tile([128, S], bf16, tag="otmP")
                ssAll = p3.tile([128, 32], fp32, tag="ssAll")
                gcrow = p3r.tile([1, S], fp32, tag="gcrow")
                nc.sync.dma_start(out=gcrow, in_=growm_d.ap()[h:h + 1, :])
                gbrdF = p3b.tile([128, S], fp32, tag="gbrdF")
                nc.gpsimd.partition_broadcast(gbrdF, gcrow)
                ngbrdF = p3b.tile([128, S], fp32, tag="ngbrdF")
                nc.vector.tensor_scalar_mul(out=ngbrdF, in0=gbrdF, scalar1=-1.0)
                Scur = p3st.tile([128, 128], bf16, tag="S")
                nc.vector.memset(Scur, 0.0)
                for c in range(NCHUNK):
                    cs = slice(c * C, (c + 1) * C)
                    col = slice(c, c + 1)
                    gbrd = gbrdF[:, cs]
                    ngbrd = ngbrdF[:, cs]
                    # decay matrices
                    dS = p3.tile([128, 128], fp32, tag="dS")
                    nc.vector.scalar_tensor_tensor(
                        out=dS, in0=ngbrd, scalar=st["gcT"][:, col], in1=maskLS,
                        op0=OP.add, op1=OP.add)
                    nc.scalar.activation(out=dS, in_=dS, func=AF.Exp)
                    dST = p3.tile([128, 128], fp32, tag="dST")
                    nc.vector.scalar_tensor_tensor(
                        out=dST, in0=gbrd, scalar=st["gcT"][:, col], in1=maskUS,
                        op0=OP.subtract, op1=OP.add)
                    nc.scalar.activation(out=dST, in_=dST, func=AF.Exp)
                    dIT = p3.tile([128, 128], fp32, tag="dIT")
                    nc.vector.scalar_tensor_tensor(
                        out=dIT, in0=gbrd, scalar=st["gcT"][:, col], in1=maskUI,
                        op0=OP.subtract, op1=OP.add)
                    nc.scalar.activation(out=dIT, in_=dIT, func=AF.Exp)
                    # KK^T and KQ^T
                    psA = ppA.tile([128, 256], fp32, tag="psA")
                    nc.tensor.matmul(psA[:, 0:128], kp[:, cs], kp[:, cs],
                                     start=True, stop=True)
                    nc.tensor.matmul(psA[:, 128:256], kp[:, cs], qp[:, cs],
                                     start=True, stop=True)
                    B0 = p3.tile([128, 128], bf16, tag="B0")
                    nc.vector.scalar_tensor_tensor(
                        out=B0, in0=psA[:, 0:128], scalar=st["nbT"][:, col],
                        in1=dS, op0=OP.mult, op1=OP.mult)
                    # transposes: B^T, K_tm, V_tm
                    psT = ppT.tile([128, 384], bf16, tag="psT")
                    nc.tensor.transpose(psT[:, 0:128], B0, identb)
                    BT0 = p3.tile([128, 128], bf16, tag="BT0")
                    nc.vector.tensor_copy(out=BT0, in_=psT[:, 0:128])
                    nc.tensor.transpose(psT[:, 128:256], kp[:, cs], identb)
                    Ktm = p3.tile([128, 128], bf16, tag="Ktm")
                    nc.vector.tensor_copy(out=Ktm, in_=psT[:, 128:256])
                    nc.tensor.transpose(psT[:, 256:384], vp[:, cs], identb)
                    X = p3s.tile([128, 256], bf16, tag="X")
                    nc.vector.tensor_scalar_mul(
                        out=X[:, 0:128], in0=Ktm, scalar1=st["bgT"][:, col])
                    nc.vector.tensor_scalar_mul(
                        out=X[:, 128:256], in0=psT[:, 256:384],
                        scalar1=st["bT"][:, col])
                    Kpr = p3.tile([128, 128], bf16, tag="Kpr")
                    nc.vector.tensor_scalar_mul(
                        out=Kpr, in0=Ktm, scalar1=st["grevT"][:, col])
                    # Neumann product chain
                    Sq, SqT = B0, BT0
                    for j in range(7):
                        psX = ppX.tile([128, 256], fp32, tag="psX")
                        nc.tensor.matmul(psX, SqT, X, start=True, stop=True)
                        Xn = p3s.tile([128, 256], bf16, tag="X")
                        nc.vector.tensor_add(out=Xn, in0=X, in1=psX)
                        X = Xn
                        if j < 6:
                            psq = ppX.tile([128, 256], fp32, tag="psq")
                            nc.tensor.matmul(psq[:, 128:256], Sq, SqT,
                                             start=True, stop=True)
                            if j < 5:
                                nc.tensor.matmul(psq[:, 0:128], SqT, Sq,
                                                 start=True, stop=True)
                                pair = p3s.tile([128, 256], bf16, tag="pair")
                                nc.vector.tensor_copy(out=pair, in_=psq)
                                Sq, SqT = pair[:, 0:128], pair[:, 128:256]
                            else:
                                nSqT = p3s.tile([128, 128], bf16, tag="nSqT")
                                nc.vector.tensor_copy(out=nSqT,
                                                      in_=psq[:, 128:256])
                                SqT = nSqT
                    # P^T, W^T P^T, H^T
                    PT = p3.tile([128, 128], bf16, tag="PT")
                    nc.vector.tensor_mul(out=PT, in0=psA[:, 128:256], in1=dIT)
                    psB = ppB.tile([128, 256], fp32, tag="psB")
                    nc.tensor.matmul(psB[:, 0:128], X[:, 0:128], PT,
                                     start=True, stop=True)
                    gamB = p3.tile([128, 128], fp32, tag="gamB")
                    nc.scalar.activation(out=gamB, in_=gbrd, func=AF.Exp)
                    QtG = p3.tile([128, 128], bf16, tag="QtG")
                    nc.vector.tensor_mul(out=QtG, in0=qp[:, cs], in1=gamB)
                    QtT = p3.tile([128, 128], bf16, tag="QtT")
                    nc.vector.tensor_sub(out=QtT, in0=QtG, in1=psB[:, 0:128])
                    nc.tensor.matmul(psB[:, 128:256], X[:, 0:128], Kpr,
                                     start=True, stop=True)
                    GT = p3.tile([128, 128], bf16, tag="GT")
                    nc.vector.scalar_tensor_tensor(
                        out=GT, in0=identf, scalar=st["gtotT"][:, col],
                        in1=psB[:, 128:256], op0=OP.mult, op1=OP.subtract)
                    # O and state update
                    psS = ppS.tile([128, 256], fp32, tag="psS")
                    nc.tensor.matmul(psS[:, 0:128], PT, X[:, 128:256],
                                     start=True, stop=False)
                    nc.tensor.matmul(psS[:, 0:128], QtT, Scur,
                                     start=False, stop=True)
                    nc.tensor.matmul(psS[:, 128:256], GT, Scur,
                                     start=True, stop=False)
                    nc.tensor.matmul(psS[:, 128:256], Kpr, X[:, 128:256],
                                     start=False, stop=True)
                    Snew = p3st.tile([128, 128], bf16, tag="S")
                    nc.vector.tensor_copy(out=Snew, in_=psS[:, 128:256])
                    # stash raw O and its row sum-of-squares; normalize later
                    osq = p3.tile([128, 128], bf16, tag="osq")
                    nc.scalar.activation(out=osq, in_=psS[:, 0:128], func=AF.Square,
                                         accum_out=ssAll[:, c:c + 1])
                    nc.vector.tensor_copy(out=otmP[:, cs], in_=psS[:, 0:128])
                    Scur = Snew
                # batched gated RMS norm + transpose + silu(z) gate
                rstdT = p3.tile([128, 32], fp32, tag="rstdT")
                nc.scalar.activation(out=rstdT, in_=ssAll, func=AF.Sqrt,
                                     bias=epsO, scale=1.0 / 128.0)
                nc.vector.reciprocal(out=rstdT, in_=rstdT)
                for c in range(NCHUNK):
                    cs = slice(c * C, (c + 1) * C)
                    otm = p3.tile([128, 128], bf16, tag="otm")
                    nc.vector.tensor_scalar_mul(out=otm, in0=otmP[:, cs],
                                                scalar1=rstdT[:, c:c + 1])
                    psTo = ppT.tile([128, 384], bf16, tag="psT")
                    psO = psTo[:, 0:128]
                    nc.tensor.transpose(psO, otm, identb)
                    nc.vector.scalar_tensor_tensor(
                        out=cp[:, cs], in0=psO, scalar=normw, in1=szp[:, cs],
                        op0=OP.mult, op1=OP.mult)
                nc.sync.dma_start(out=core_d[h * 128:h * 128 + 128, :], in_=cp)
                if debug:
                    nc.sync.dma_start(
                        out=dbg["d_core"].ap()[h * 128:h * 128 + 128, :], in_=cp)

        # ---------------- Phase 4: out projection ----------------
        with tc.tile_pool(name="p4w", bufs=1) as p4w, \
             tc.tile_pool(name="p4", bufs=3) as p4, \
             tc.tile_pool(name="pp4", bufs=4, space="PSUM") as pp4:
            wot = p4w.tile([128, HPC, D], bf16, tag="wot")
            nc.sync.dma_start(
                out=wot, in_=wog.ap().rearrange("(j p) d -> p j d", p=128))
            for nt in range(NT):
                ts = slice(nt * 512, (nt + 1) * 512)
                cblk = p4.tile([128, HPC, 512], bf16, tag="cblk")
                nc.sync.dma_start(
                    out=cblk,
                    in_=core_d.rearrange("(j p) t -> p j t", p=128)[:, :, ts])
                for dt in range(16):
                    ps = pp4.tile([128, 512], fp32, tag="ps")
                    for j in range(HPC):
                        nc.tensor.matmul(ps, wot[:, j, dt * 128:(dt + 1) * 128],
                                         cblk[:, j, :],
                                         start=(j == 0), stop=(j == HPC - 1))
                    ot = p4.tile([128, 512], bf16, tag="ot")
                    nc.scalar.activation(out=ot, in_=ps, func=AF.Copy)
                    nc.sync.dma_start(out=rs_in.ap()[dt * 128:(dt + 1) * 128, ts],
                                      in_=ot)

        nc.gpsimd.collective_compute(
            "ReduceScatter", mybir.AluOpType.add, replica_groups=G4,
            ins=[rs_in.ap()], outs=[rs_out.ap()])
        with tc.tile_pool(name="fin", bufs=2) as fin:
            for j in range(4):
                t = fin.tile([128, S], bf16, tag="fo")
                nc.sync.dma_start(out=t, in_=rs_out.ap()[j * 128:(j + 1) * 128, :])
                nc.sync.dma_start(out=outR.ap()[j * 128:(j + 1) * 128, :], in_=t)

    nc.compile()
    return nc


def _host_inputs(hidden_states, W_qkv, W_z, W_b, W_a, conv_w, A_log, dt_bias,
                 norm_w, W_out):
    """Per-core input maps.  beta/g decay stats computed on host (tiny)."""
    masks = np.zeros((128, 384), np.float32)
    r = np.arange(128)
    masks[:, 0:128] = np.where(r[None, :] < r[:, None], 0.0, NEG)     # s<t
    masks[:, 128:256] = np.where(r[None, :] > r[:, None], 0.0, NEG)   # f>p
    masks[:, 256:384] = np.where(r[None, :] >= r[:, None], 0.0, NEG)  # f>=p
    identity = np.eye(128, dtype=np.float32)

    hs2 = hidden_states.reshape(B * S, D).astype(np.float32)
    bet = 1.0 / (1.0 + np.exp(-(hs2 @ W_b)))                    # [B*S, 16]
    g = (-np.exp(A_log.astype(np.float32))[None, :]
         * np.logaddexp(0.0, hs2 @ W_a + dt_bias[None, :]))     # [B*S, 16]
    bet = bet.reshape(B, NCHUNK, C, HK)
    g = g.reshape(B, NCHUNK, C, HK)
    gc_all = np.cumsum(g, axis=2)                               # [B, nc, C, H]

    in_maps = []
    for c in range(8):
        b = c // 4
        h0 = 4 * (c % 4)
        ch = slice(h0 * 128, (h0 + 4) * 128)
        W_all = np.concatenate([
            W_qkv[:, h0 * 128:(h0 + 4) * 128],
            W_qkv[:, KEY_DIM + h0 * 128:KEY_DIM + (h0 + 4) * 128],
            W_qkv[:, 2 * KEY_DIM + h0 * 128:2 * KEY_DIM + (h0 + 4) * 128],
            W_z[:, h0 * 128:(h0 + 4) * 128],
        ], axis=1).astype(BF16)
        cw = np.concatenate([
            conv_w[h0 * 128:(h0 + 4) * 128, 0, :],
            conv_w[KEY_DIM + h0 * 128:KEY_DIM + (h0 + 4) * 128, 0, :],
            conv_w[2 * KEY_DIM + h0 * 128:2 * KEY_DIM + (h0 + 4) * 128, 0, :],
        ], axis=0).astype(np.float32)

        growm = np.zeros((2 * HPC, S), np.float32)
        colst = np.zeros((128, HPC * 192), np.float32)
        for j in range(HPC):
            gc = gc_all[b, :, :, h0 + j]                        # [nc, C]
            be = bet[b, :, :, h0 + j]
            gam = np.exp(gc)
            growm[j] = gc.reshape(S)
            growm[HPC + j] = -gc.reshape(S)
            o = j * 192
            colst[:, o:o + 32] = gc.T                           # gcT
            colst[:, o + 32:o + 64] = be.T                      # bT
            colst[:, o + 64:o + 96] = -be.T                     # nbT
            colst[:, o + 96:o + 128] = np.exp(gc[:, -1][None, :] - gc.T)  # grevT
            colst[:, o + 128:o + 160] = (be * gam).T            # bgT
            colst[:, o + 160:o + 192] = np.exp(gc[:, -1])[None, :] * np.ones((128, 1), np.float32)  # gtotT
        q4 = c % 4
        half = slice(0, D // 2) if b == 0 else slice(D // 2, D)
        oh = slice(h0 * 128, h0 * 128 + 256) if b == 0 else \
             slice(h0 * 128 + 256, (h0 + 4) * 128)
        in_maps.append({
            "hsQ": np.ascontiguousarray(
                hidden_states[b, q4 * (S // 4):(q4 + 1) * (S // 4), :].T
                ).astype(BF16),
            "W_half": np.ascontiguousarray(W_all[half, :]),
            "Wo_half": W_out[oh, :].astype(BF16),
            "convw": np.ascontiguousarray(cw),
            "growm": growm,
            "colstats": colst,
            "normw": norm_w.astype(np.float32).reshape(128, 1),
            "masks": masks,
            "identb": identity.astype(BF16),
            "identf": identity,
        })
    return in_maps


def _setup_jax_cache():
    try:
        import jax
        cache_dir = "/var/tmp/jaxcache"
        os.makedirs(cache_dir, exist_ok=True)
        jax.config.update("jax_compilation_cache_dir", cache_dir)
        jax.config.update("jax_persistent_cache_min_entry_size_bytes", 0)
        jax.config.update("jax_persistent_cache_min_compile_time_secs", 0.0)
    except Exception:
        pass


def kernel(hidden_states, W_qkv, W_z, W_b, W_a, conv_w, A_log, dt_bias,
           norm_w, W_out):
    from concourse import bass_utils

    _setup_jax_cache()
    if "nc" not in _CACHE:
        _CACHE["nc"] = build_nc(debug=False)
    nc = _CACHE["nc"]
    in_maps = _host_inputs(hidden_states, W_qkv, W_z, W_b, W_a, conv_w,
                           A_log, dt_bias, norm_w, W_out)
    res = bass_utils.run_bass_kernel_spmd(nc, in_maps, core_ids=list(range(8)))
    out = np.empty((B, S, D), np.float32)
    for b in range(B):
        outT = np.concatenate(
            [res.results[b * 4 + r]["outR"] for r in range(4)], axis=0)
        out[b] = outT.astype(np.float32).T
    return out
